# revision 77
# baseline (speedup 1.0000x reference)
"""MultiHeadExternalAttention Trainium2 kernel (v3).

Math (reference):
  h = x @ trans_w.T + trans_b            [B,N,4096] -> heads [B,64,N,64]
  a = h @ lin0_w.T + lin0_b              per-head [B,64,N,64]
  a = softmax(a, axis=N)
  a = a / (1e-10 + a.sum(-1, keepdims))  double norm over j
  o = a @ lin1_w.T + lin1_b
  out = o (merged heads) @ proj_w.T + proj_b

Both tiny linears fold into the big matmuls on the host:
  logits[b,h,n,j] = x[b,n,:] @ fw[h,j,:] + fb[h,j]     fw = lin0_w @ trans_w_h
  out[b,n,c]     += attn[b,h,n,k] * g[h,c,k]           g  = proj_w_h @ lin1_w
  out[b,n,c]     += cb[c]                               (host)

Sharding: 8 cores = 4 batches x 2 head-halves (32 heads = 16 head-pairs per
core). v2: host-pretransposed bf16 xT; column-tiled jsum matmuls; DMA-based
r-broadcast; bf16 output partials.

v3 changes (TimelineSim 543 -> 527 us; each validated against the
device-occupancy simulator):
  - startup: hp0+hp1 logits emission token-interleaved so the PE stream
    matches x-chunk arrival order (hp-serial emission head-of-line blocked
    the PE on hp0's late windows: ~17 us idle).
  - softmax column scale rs folds into the phase-2 weights on device
    (p2' = rs*p2 per hp, 16 cheap DVE ops), so the attn scaling pass is a
    plain tensor_tensor (DVE 2x_1p mode, 297 ns/window) instead of
    scalar_tensor_tensor (no 2x support, 594 ns/window): DVE busy 197 ->
    132 us in sim, and half the STT time on HW.
  - attn spills to DRAM scratch go out piecewise (per 4 STT windows) on the
    sync queue: the a2 gathers depend only on the piece covering their
    tokens, and the Q7 descriptor-gen path is avoided.
  - the a2 gather is split (hp 0..12 / hp 13) so early-chunk matmuls don't
    wait on the last head-pair's spill.
  - transition: the first 2 phase-2 chunks emit their hp0..12 matmuls ahead
    of jsum(15) (whose Tmh dependency would head-of-line block the 4-deep
    PE wait queue), with the hp13/14/15 finishers deferred; the tail jsums
    run q-major ("pipelined") with per-q psum tiles from the idle p1p ring,
    per-q reciprocals, rdram stores and RB broadcast pieces, and the tail
    STT starts right behind them.
  - Tmh is allocated+cleared at slab start and jsum emission is deferred
    two s1 yields so its ldweights never clogs the PE wait queue.
  - p2 loads: slices 0:2 mid-x-stream, rest at epoch 2 (a bulk load would
    starve the startup x stream; per-hp slices later would pin DVE
    wait-queue slots on the in-flight rs folds).

Numerics note: RECIPROCAL_APPROX_FAST with a bf16 output produces NaNs on
HW (the f32-out assert in bass.py is load-bearing); the exact DVE
reciprocal stays.
"""

import sys

if "/opt/trn_rl_repo" not in sys.path:
    sys.path.insert(0, "/opt/trn_rl_repo")

import numpy as np
import ml_dtypes

import concourse.bass as bass
import concourse.bacc as bacc
import concourse.mybir as mybir
import concourse.tile as tile

BF16NP = ml_dtypes.bfloat16
F32 = mybir.dt.float32
BF = mybir.dt.bfloat16
AF = mybir.ActivationFunctionType

DIM = 512
HEADS = 64
K = 64
B = 4
NTOK = 8192
NCORES = 8
HPC = 16  # head pairs per core

USE_DMA_BCAST = True
# The bf16-out RECIPROCAL_APPROX_FAST produces NaNs on HW (the f32-out
# assert in bass.py is load-bearing); keep the exact DVE reciprocal.
USE_APPROX_RECIP = False
PREF_N = 3
A2P_N = 3
JSP_N = 1
# Compute the first 128 of the 512-wide logits contraction as one fp8
# DoubleRow matmul (2x64 packed rows): 3 bf16 + 1 DR matmul per window
# instead of 4 bf16. Numerically fine (1.49e-2 vs the 2e-2 gate) but the
# DoubleRow weight loads defeat the LDW pipelining on HW: measured 589 us
# vs 471 us without. Keep off.
FP8_QUARTER = False
F8 = mybir.dt.float8e4
F8NP = ml_dtypes.float8_e4m3


def build_bass(ntok=NTOK, n_hp=HPC, reps=1):
    nc = bacc.Bacc()
    NCC = 3 if FP8_QUARTER else 4
    xT_d = nc.dram_tensor("xT", [128, NCC, ntok], BF, kind="ExternalInput")
    w2 = nc.dram_tensor("w2", [128, n_hp, NCC, 128], BF, kind="ExternalInput")
    if FP8_QUARTER:
        xT8_d = nc.dram_tensor("xT8", [64, 2, ntok], F8, kind="ExternalInput")
        w28 = nc.dram_tensor("w28", [64, n_hp, 2, 128], F8, kind="ExternalInput")
    else:
        xT8_d = w28 = None
    c1 = nc.dram_tensor("c1", [128, n_hp], F32, kind="ExternalInput")
    p2 = nc.dram_tensor("p2", [128, n_hp, DIM], BF, kind="ExternalInput")
    d2 = nc.dram_tensor("d2", [128, 128], BF, kind="ExternalInput")
    out_p = nc.dram_tensor("out_p", [ntok, DIM], BF, kind="ExternalOutput")

    with tile.TileContext(nc) as tc:
        with tc.tile_pool(name="const", bufs=1) as const, tc.tile_pool(
            name="dramp", bufs=1, space="DRAM"
        ) as dramp:
            scratch = dramp.tile([max(n_hp - 2, 1), 128, ntok], BF)

            w2_sb = const.tile([128, n_hp, NCC, 128], BF)
            c1_sb = const.tile([128, n_hp], F32)
            p2_sb = const.tile([128, n_hp, DIM], BF)
            xT = const.tile([128, NCC, ntok], BF)
            if FP8_QUARTER:
                w28_sb = const.tile([64, n_hp, 2, 128], F8)
                xT8 = const.tile([64, 2, ntok], F8)
            else:
                w28_sb = xT8 = None
            D2 = const.tile([128, 128], BF)
            if not USE_DMA_BCAST:
                nc.scalar.dma_start(out=D2, in_=d2[:])

            for _rep in range(reps):
                run_pipeline(
                    nc, tc, xT_d, out_p, scratch, w2_sb, c1_sb, p2_sb, xT, D2,
                    dramp, ntok, n_hp,
                    (w2, c1, p2, w28) if _rep == 0 else None,
                    xT8_d=xT8_d, w28_sb=w28_sb, xT8=xT8,
                )
    nc.finalize()
    return nc


def run_pipeline(
    nc, tc, xT_d, out_p, scratch, w2_sb, c1_sb, p2_sb, xT, D2, dramp, ntok,
    n_hp, dram_consts=None, xT8_d=None, w28_sb=None, xT8=None,
):
    NT = ntok // 128
    NW = ntok // 512
    Q = min(4, NW)
    NS = NW // Q

    if dram_consts is not None:
        w2d, c1d, p2d, w28d = dram_consts
        # hp0 weight slice + bias first so the first logits matmul waits
        # only on 128KB of weights + one x chunk.
        nc.scalar.dma_start(out=w2_sb[:, 0:1], in_=w2d[:, 0:1])
        nc.scalar.dma_start(out=w2_sb[:, 1:2], in_=w2d[:, 1:2])
        if FP8_QUARTER:
            nc.scalar.dma_start(out=w28_sb, in_=w28d[:])
        nc.scalar.dma_start(out=c1_sb, in_=c1d[:])
    # x load in chunks so phase 1 can start early (deps are AP-granular).
    # All x loads on the sync HWDGE queue. Measured dead ends: gpsimd
    # routing for rep>=2 (Q7 descriptor-gen ~9us per 512-descriptor strided
    # chunk: 471->608us), scalar routing (x queues behind the final out
    # stores, no overlap gained).
    xq = nc.sync
    NXCH = 16
    for i in range(NXCH):
        lo, hi = ntok * i // NXCH, ntok * (i + 1) // NXCH
        xq.dma_start(out=xT[:, :, lo:hi], in_=xT_d[:, :, lo:hi])
        if FP8_QUARTER:
            xq.dma_start(out=xT8[:, :, lo:hi], in_=xT8_d[:, :, lo:hi])
        if i == 0 and dram_consts is not None:
            nc.scalar.dma_start(out=w2_sb[:, 2:n_hp], in_=w2d[:, 2:n_hp])
        if i == 11 and dram_consts is not None:
            # p2 slices for hp0/1 must be resident before emit_jsum folds rs
            # into them at the end of the interleaved startup (~13us); the
            # rest loads after the x stream so it can't starve it
            nc.scalar.dma_start(out=p2_sb[:, 0:2], in_=p2d[:, 0:2])

    # 4 tail head-pairs stay SBUF-resident: the ep pool is a 4-buffer ring
    # and nothing re-allocates after e(n_hp-4)..e(n_hp-1), so those slabs
    # survive into phase 2 for free — spilling them (and gathering them
    # back through the transition's saturated DMA window) was pure waste.
    ns = max(n_hp - 4, 0)
    tail_attn = {}
    # PSUM budget is 8 banks: p1p + jsp + op + (scp only for the PE-bcast
    # fallback; with DMA bcast its banks go to deeper logits buffering).
    n_p1 = 4 if USE_DMA_BCAST else 3
    import contextlib
    scp_ctx = (
        contextlib.nullcontext(None)
        if USE_DMA_BCAST
        else tc.tile_pool(name="scp", bufs=2, space="PSUM")
    )
    with tc.tile_pool(
        name="p1p", bufs=n_p1, space="PSUM"
      ) as p1p, tc.tile_pool(
        name="jsp", bufs=JSP_N, space="PSUM"
      ) as jsp, scp_ctx as scp, tc.tile_pool(name="ep", bufs=4) as ep, tc.tile_pool(
        name="rp", bufs=2
      ) as rp, tc.tile_pool(name="tmp", bufs=2) as tmp, tc.tile_pool(
        name="small", bufs=4
      ) as small, tc.tile_pool(name="a2p", bufs=A2P_N) as a2p, tc.tile_pool(
        # op stays at 2 psum banks: 3 banks measured 582us vs 471us (the
        # documented psum-bank-cycling HAM oscillation trap)
        name="op", bufs=2, space="PSUM"
      ) as op, tc.tile_pool(name="osp", bufs=2) as osp, tc.tile_pool(
        name="rbp", bufs=2
      ) as rbp:
        state = {}

        def gen_step1(hp, ygran=4, w28_sb=w28_sb, xT8=xT8):
            e = ep.tile([128, ntok], BF, name="e")
            scol = small.tile([128, NW], F32, name="scol")
            state[hp] = [e, None, None]
            # allocate + clear Tmh up front: only the rs copies depend on
            # the finished slab, so the DVE memset is long done by the time
            # emit_jsum's ldweights needs the tile (PE wait-queue is only 4
            # deep — a not-ready lhsT clogs it for unrelated matmuls).
            Tmh = tmp.tile([128, 66], BF, name="Tmh")
            nc.vector.memset(Tmh, 0.0)
            NCC = 3 if FP8_QUARTER else 4
            for t8 in range(NW):
                p1 = p1p.tile([128, 512], F32, name="p1")
                base = 512 * t8
                if FP8_QUARTER:
                    nc.tensor.matmul(
                        p1,
                        lhsT=w28_sb[:, hp],
                        rhs=xT8[:, :, base : base + 512],
                        perf_mode=mybir.MatmulPerfMode.DoubleRow,
                        start=True,
                        stop=False,
                    )
                for cc in range(NCC):
                    nc.tensor.matmul(
                        p1,
                        lhsT=w2_sb[:, hp, cc, :],
                        rhs=xT[:, cc, base : base + 512],
                        start=(cc == 0 and not FP8_QUARTER),
                        stop=(cc == NCC - 1),
                    )
                nc.scalar.activation(
                    e[:, base : base + 512],
                    p1,
                    func=AF.Exp,
                    bias=c1_sb[:, hp : hp + 1],
                    scale=1.0,
                    accum_out=scol[:, t8 : t8 + 1],
                )
                if t8 % ygran == ygran - 1:
                    yield
            s1 = small.tile([128, 1], F32, name="s1")
            nc.vector.reduce_sum(s1, scol, axis=mybir.AxisListType.X)
            rs = small.tile([128, 1], F32, name="rs")
            nc.vector.reciprocal(rs, s1)
            nc.vector.tensor_copy(Tmh[0:64, 32:33], rs[0:64, :])
            nc.vector.tensor_copy(Tmh[64:128, 33:34], rs[64:128, :])
            state[hp][1] = rs
            state[hp][2] = Tmh

        def emit_jsum(hp, pipelined=False):
            e, rs, Tmh = state[hp]
            assert Tmh is not None
            # 4-way column-tiled partition sums: window w = NS*q + s runs in
            # col-group q, accumulating at psum rows 32q + 2s + g. Body hps
            # emit s-major (consecutive matmuls in distinct col groups run
            # concurrently, recips/stores batched after); the tail hps emit
            # q-major (pipelined=True) so group q's accumulation closes
            # after its 4 matmuls, letting recip(q) -> rdram(q) -> RB
            # piece(q) -> STT windows 4q..4q+4 pipeline against group q+1's
            # matmuls — the whole r apparatus for the last hp sits on the
            # phase-1 -> phase-2 critical path.
            # pipelined mode allocates one PSUM tile per q-group from p1p
            # (idle by the transition): with a single shared js tile the
            # dependency tracking serializes group q+1's matmuls behind
            # recip(q)'s read (tile-granular WAR).
            js = None if pipelined else jsp.tile([128, 512], F32, name="js")
            jrb = rp.tile([128, 512], BF, name="jrb")
            # rdram row = 2w + g (window-ordered: rows 8q+2s+g, w = NS*q+s)
            rdram = dramp.tile([2 * NW, 512], BF, name="rdram", bufs=2)
            RB = rbp.tile([128, NW, 512], BF, name="RB") if USE_DMA_BCAST else None
            r2a = None if USE_DMA_BCAST else rp.tile([128, NS, 512], BF, name="r2a")
            rvb = rdram.rearrange("(w g) n -> g w n", w=NW, g=2)
            rva = rdram.rearrange("(k q g) n -> q g k n", k=NS, q=Q)
            from concourse.dve_ops import (
                RECIP_APPROX_FAST_CONSTS,
                RECIPROCAL_APPROX_FAST,
            )

            cns = RECIP_APPROX_FAST_CONSTS
            if not pipelined:
                for s in range(NS):
                    for q in range(Q):
                        w = NS * q + s
                        nc.tensor.matmul(
                            js[32 * q : 32 * q + 32, :],
                            lhsT=Tmh[:, 32 - 2 * s : 64 - 2 * s],
                            rhs=e[:, 512 * w : 512 * (w + 1)],
                            start=(s == 0),
                            stop=(s == NS - 1),
                            tile_position=(0, 32 * q),
                        )
            for q in range(Q):
                if pipelined:
                    js = p1p.tile([128, 512], F32, name="p1")
                    for s in range(NS):
                        w = NS * q + s
                        nc.tensor.matmul(
                            js[32 * q : 32 * q + 32, :],
                            lhsT=Tmh[:, 32 - 2 * s : 64 - 2 * s],
                            rhs=e[:, 512 * w : 512 * (w + 1)],
                            start=(s == 0),
                            stop=(s == NS - 1),
                            tile_position=(0, 32 * q),
                        )
                with nc.allow_low_precision(reason="r broadcast is bf16 either way"):
                    if USE_APPROX_RECIP:
                        # ~51-ULP approximate reciprocal (1 DVE op, ~1
                        # cyc/elem vs 6 for the exact iterative divide on
                        # HW). Emitted via _custom_dve to allow the bf16
                        # output: the fp32 requirement is about the
                        # BITWISE_NOT seed reading fp32 bits — the INPUT
                        # (psum f32) — while the write port downcasts like
                        # any DVE op. Denominators are sums of positives, far
                        # from the undefined 0/denorm/inf edges.
                        nc.vector._custom_dve(
                            RECIPROCAL_APPROX_FAST,
                            out=jrb[32 * q : 32 * q + 2 * NS, :],
                            in0=js[32 * q : 32 * q + 2 * NS, :],
                            s0=cns["s0"],
                            s1=cns["s1"],
                            imm2=cns["imm2"],
                        )
                    else:
                        nc.vector.reciprocal(
                            jrb[32 * q : 32 * q + 2 * NS, :],
                            js[32 * q : 32 * q + 2 * NS, :],
                        )
                nc.scalar.dma_start(
                    out=rdram[2 * NS * q : 2 * NS * (q + 1), :],
                    in_=jrb[32 * q : 32 * q + 2 * NS, :],
                )
                if USE_DMA_BCAST:
                    # replicating read: RB[64g+k, w, nn] = rdram[2w+g, nn],
                    # piece q covers windows 4q..4q+4 (rdram rows 8q..8q+8,
                    # exactly the q-store above). sync-queue HWDGE: the
                    # descriptor expansion would swamp the gpsimd Q7
                    # generator, and the scalar queue head-of-line blocks the
                    # exp activations behind the rdram wait.
                    for g in range(2):
                        nc.sync.dma_start(
                            out=RB[64 * g : 64 * (g + 1), NS * q : NS * (q + 1), :],
                            in_=rvb[g][NS * q : NS * (q + 1)].partition_broadcast(64),
                        )
                else:
                    # pack window w at partitions 32*(w%Q)+{0,1}, free slot
                    # w//Q
                    nc.sync.dma_start(
                        out=r2a[32 * q : 32 * q + 2, :, :], in_=rva[q]
                    )
            # fold the softmax column scale rs into this hp's phase-2
            # weights: attn is then stored as e*rjs (a plain tensor_tensor,
            # which the DVE runs in 2x mode — scalar_tensor_tensor cannot)
            # and out = sum_j (e*rjs)[j,n] * (rs_j*p2[j,c]) is unchanged.
            nc.vector.tensor_scalar(
                out=p2_sb[:, hp, :],
                in0=p2_sb[:, hp, :],
                scalar1=rs,
                scalar2=None,
                op0=mybir.AluOpType.mult,
            )
            state[hp] = [e, rs, RB if USE_DMA_BCAST else r2a]

        def gen_stt(hp):
            e, rs, rmat = state.pop(hp)
            attn = e  # in-place: attn overwrites the e slab window by window
            if hp >= ns:
                tail_attn[hp] = attn
            for w in range(NW):
                if USE_DMA_BCAST:
                    in1 = rmat[:, w, :]
                else:
                    q = w % Q
                    sc = scp.tile([128, 512], F32, name="sc")
                    nc.tensor.matmul(
                        sc,
                        lhsT=D2[32 * q : 32 * q + 2, :],
                        rhs=rmat[32 * q : 32 * q + 2, w // Q, :],
                        start=True,
                        stop=True,
                        tile_position=(32 * q, 0),
                    )
                    in1 = sc
                nc.vector.tensor_tensor(
                    out=attn[:, 512 * w : 512 * (w + 1)],
                    in0=e[:, 512 * w : 512 * (w + 1)],
                    in1=in1,
                    op=mybir.AluOpType.mult,
                )
                if w % 4 == 3:
                    # piecewise spill on the sync HWDGE queue: each a2 gather
                    # for token chunk i then depends only on the piece
                    # covering its tokens, not the whole-slab spill — keeps
                    # the phase-1 -> phase-2 transition off the Q7
                    # descriptor-gen path and off the whole-slab dependency.
                    if hp < ns:
                        lo, hi = 512 * (w - 3), 512 * (w + 1)
                        nc.sync.dma_start(
                            out=scratch[hp][:, lo:hi], in_=attn[:, lo:hi]
                        )
                    yield

        # phase 2: out[n,c] = sum_hp attn_hp[:, chunk].T @ p2_hp
        PREF = PREF_N

        def issue_a2_a(i):
            # part A: hp 0..ns-2, spilled early — issued one epoch before
            # the transition so the big gathers (~2.4us each on the
            # exclusive DMA engines) are out of the transition's DMA crunch
            a2 = a2p.tile([128, ns, 128], BF, name="a2")
            if ns > 1:
                nc.sync.dma_start(
                    out=a2[:, 0 : ns - 1],
                    in_=scratch[
                        0 : ns - 1, :, 128 * i : 128 * (i + 1)
                    ].rearrange("h p n -> p h n"),
                )
            return a2

        def issue_a2_b(i, a2):
            # part B: hp ns-1, whose spill lands last. MUST be emitted after
            # that spill's first piece (same sync queue: a not-ready DMA at
            # the queue head would deadlock against the piece queued behind
            # it).
            nc.sync.dma_start(
                out=a2[:, ns - 1 : ns],
                in_=scratch[
                    ns - 1 : ns, :, 128 * i : 128 * (i + 1)
                ].rearrange("h p n -> p h n"),
            )

        def issue_a2(i):
            a2 = issue_a2_a(i)
            issue_a2_b(i, a2)
            return a2

        def gen_phase2_prefetch_a(a2s):
            if ns > 1:
                for i in range(min(PREF, NT)):
                    a2s[i] = issue_a2_a(i)
                    yield

        def gen_phase2_prefetch(a2s):
            if ns > 0:
                for i in range(min(PREF, NT)):
                    if i not in a2s:
                        a2s[i] = issue_a2_a(i)
                    issue_a2_b(i, a2s[i])
                    yield

        # first S2 chunks are emitted in two parts: the hp0..n_hp-2 matmuls
        # go into the PE stream BEFORE the last hp's jsum (they only need a2
        # + the first STT window of tail hp n_hp-2, all ready while the last
        # hp's r apparatus resolves), and the hp n_hp-1 finisher is emitted
        # with the tail STT. This keeps the PE fed across the phase-1 ->
        # phase-2 transition instead of head-of-line blocking on jsum(last).
        S2 = 2 if (ns >= 2 and n_hp >= 4) else 0
        p2heads = {}

        def gen_phase2_head(a2s):
            for i in range(S2):
                a2 = a2s.pop(i)
                po = op.tile([128, 512], F32, name="po")
                # hp 0..ns-2 only: everything already spilled by the end of
                # epoch n_hp-1 (the late hp ns-1 spill and the tail STTs are
                # covered by the finisher)
                for hpi in range(ns - 1):
                    nc.tensor.matmul(
                        po,
                        lhsT=a2[:, hpi, :],
                        rhs=p2_sb[:, hpi, :],
                        start=(hpi == 0),
                        stop=False,
                    )
                p2heads[i] = (po, a2)
                yield

        def gen_phase2(a2s):
            for i in range(NT):
                if i in p2heads:
                    po, a2 = p2heads.pop(i)
                    for hpi in range(ns - 1, n_hp):
                        if hpi < ns:
                            lhsT = a2[:, hpi, :]
                        else:
                            lhsT = tail_attn[hpi][:, 128 * i : 128 * (i + 1)]
                        nc.tensor.matmul(
                            po,
                            lhsT=lhsT,
                            rhs=p2_sb[:, hpi, :],
                            start=False,
                            stop=(hpi == n_hp - 1),
                        )
                else:
                    if ns > 0:
                        a2 = a2s.pop(i)
                    po = op.tile([128, 512], F32, name="po")
                    for hpi in range(n_hp):
                        if hpi < ns:
                            lhsT = a2[:, hpi, :]
                        else:
                            lhsT = tail_attn[hpi][:, 128 * i : 128 * (i + 1)]
                        nc.tensor.matmul(
                            po,
                            lhsT=lhsT,
                            rhs=p2_sb[:, hpi, :],
                            start=(hpi == 0),
                            stop=(hpi == n_hp - 1),
                        )
                if ns > 0 and i + PREF < NT:
                    # emitted after this chunk's matmuls: the new a2 reuses
                    # the buffer those matmuls are still reading
                    a2s[i + PREF] = issue_a2(i + PREF)
                osb = osp.tile([128, DIM], BF, name="osb")
                nc.scalar.activation(osb, po, func=AF.Copy)
                nc.scalar.dma_start(
                    out=out_p[128 * i : 128 * (i + 1), :], in_=osb
                )
                yield

        a2s = {}
        stt_started = {}
        p2preA = gen_phase2_prefetch_a(a2s)
        p2pre = gen_phase2_prefetch(a2s)
        p2head = gen_phase2_head(a2s)
        p2g = gen_phase2(a2s)
        # Startup: token-interleaved emission of hp0+hp1 logits so the PE
        # instruction stream consumes windows in x-chunk arrival order
        # (hp-serial emission head-of-line-blocks the PE on hp0's late
        # windows while hp1's early windows already have data).
        if n_hp >= 2:
            g0, g1 = gen_step1(0, ygran=1), gen_step1(1, ygran=1)
            d0 = d1 = False
            while not (d0 and d1):
                if not d0 and next(g0, "END") == "END":
                    d0 = True
                    emit_jsum(0)
                if not d1 and next(g1, "END") == "END":
                    d1 = True
            hp_start = 2
        else:
            hp_start = 0
        for hp in range(hp_start, n_hp + 2):
            if hp == hp_start and dram_consts is not None:
                # the rest of p2 loads once the x stream is done competing
                # for the DMA engines (needed from jsum(2) ~epoch 3 on)
                nc.scalar.dma_start(out=p2_sb[:, 2:n_hp], in_=dram_consts[2][:, 2:n_hp])
            s1g = gen_step1(hp) if hp < n_hp else None
            # countdown: emit jsum(hp-1) only after two s1 yields (~8 logits
            # windows), by which point the DVE has drained the Tmh rs-copies
            # — otherwise the jsum matmuls sit not-ready in the 4-deep PE
            # wait queue and block the logits stream behind them.
            jsum_count = 2 if max(hp_start, 1) <= hp <= n_hp else -1
            if hp == n_hp and S2 > 0:
                # transition epoch: the S2 chunk heads were already emitted
                # at the end of the previous epoch; start the tail STT
                # (registers its slab + emits window 0) right behind
                # jsum(last).
                bcg = gen_stt(hp - 2)
                next(bcg, None)
                emit_jsum(hp - 1, pipelined=True)
                jsum_count = -1
                # start the tail STT immediately so its first windows land in
                # the DVE queue ahead of STT(hp-2)'s remaining windows (the
                # chunk finishers and early full chunks only need tail
                # windows 0..3)
                g_tail = gen_stt(hp - 1)
                next(g_tail, None)
                stt_started[hp - 1] = g_tail
            else:
                if jsum_count >= 0 and s1g is None:
                    emit_jsum(hp - 1, pipelined=(hp - 1 >= n_hp - 2))
                    jsum_count = -1
                if hp - 2 in stt_started:
                    bcg = stt_started.pop(hp - 2)
                else:
                    bcg = gen_stt(hp - 2) if hp >= 2 else None
            # prefetch epoch starts one earlier than phase 2: a2 gathers
            # interleave with the STT spill pieces they depend on; last
            # epoch: full phase-2 chunks.
            preA = hp >= n_hp - 2
            # all spills (hp < ns = n_hp-4) complete two epochs before the
            # transition, so the full prefetch can run at n_hp-2 as well
            pre, tail = hp >= n_hp - 2, hp == n_hp + 1
            while s1g is not None or bcg is not None:
                if s1g is not None and next(s1g, "END") == "END":
                    s1g = None
                if jsum_count >= 0:
                    jsum_count -= 1
                    if jsum_count <= 0 or s1g is None:
                        emit_jsum(hp - 1, pipelined=(hp - 1 >= n_hp - 2))
                        jsum_count = -1
                if bcg is not None and next(bcg, "END") == "END":
                    bcg = None
                if preA:
                    next(p2preA, None)
                if pre:
                    next(p2pre, None)
                if tail:
                    next(p2g, None)
            if hp == n_hp - 1 and S2 > 0:
                # emit the S2 chunk heads (hp 0..ns-2 matmuls) at the end of
                # this epoch: their a2 gathers and spills are complete, so
                # they give the PE covering work while the last hp's jsum/r
                # apparatus resolves next epoch
                for _ in range(S2):
                    next(p2pre, None)  # ensure a2[0..S2-1] issued
                for _ in range(S2):
                    next(p2head, None)
        for _ in p2preA:
            pass
        for _ in p2pre:
            pass
        for _ in p2head:
            pass
        for _ in p2g:
            pass
    tail_attn.clear()


def fuse_weights(inputs):
    tw = np.asarray(inputs["trans_w"], np.float64)  # [4096, 512]
    tb = np.asarray(inputs["trans_b"], np.float64)  # [4096]
    l0w = np.asarray(inputs["lin0_w"], np.float64)  # [64, 64]
    l0b = np.asarray(inputs["lin0_b"], np.float64)
    l1w = np.asarray(inputs["lin1_w"], np.float64)
    l1b = np.asarray(inputs["lin1_b"], np.float64)
    pw = np.asarray(inputs["proj_w"], np.float64)  # [512, 4096]
    pb = np.asarray(inputs["proj_b"], np.float64)

    tw3 = tw.reshape(HEADS, K, DIM)
    tb2 = tb.reshape(HEADS, K)
    fw = np.einsum("jk,hkc->hjc", l0w, tw3)  # [64, 64, 512]
    fb = l0b[None, :] + np.einsum("jk,hk->hj", l0w, tb2)  # [64, 64]
    pw3 = pw.reshape(DIM, HEADS, K).transpose(1, 0, 2)  # [h, c, j]
    g = np.einsum("hcj,jk->hck", pw3, l1w)  # [64, 512, 64]
    cb = pb + np.einsum("hcj,j->c", pw3, l1b)  # [512]
    return fw, fb, g, cb


def make_xt(xb):
    """xT[128, 4, ntok] bf16 from x[b] [ntok, DIM] f32."""
    ntok = xb.shape[0]
    xt = np.asarray(xb, np.float32).T.astype(BF16NP)  # [512, ntok]
    return np.ascontiguousarray(
        xt.reshape(4, 128, ntok).transpose(1, 0, 2)
    )


def make_core_inputs(x, fw, fb, g, b, gg, n_hp=HPC, xt_cache=None):
    """Inputs for the core handling batch b, head half gg (heads 32*gg..+32)."""
    h0 = (HEADS // 2) * gg
    w2 = np.empty((128, n_hp, 4, 128), BF16NP)
    c1 = np.empty((128, n_hp), np.float32)
    p2 = np.empty((128, n_hp, DIM), BF16NP)
    for hp in range(n_hp):
        ha, hb = h0 + 2 * hp, h0 + 2 * hp + 1
        blk = np.concatenate([fw[ha], fw[hb]], axis=0)  # [128 j2, 512 c]
        # w2[ci, hp, cc, j2] = blk[j2, cc*128+ci]
        w2[:, hp, :, :] = blk.reshape(128, 4, 128).transpose(2, 1, 0).astype(BF16NP)
        c1[:, hp] = np.concatenate([fb[ha], fb[hb]]).astype(np.float32)
        # p2[g2*64+k, hp, c] = g[head, c, k]
        p2[0:64, hp, :] = g[ha].T.astype(BF16NP)
        p2[64:128, hp, :] = g[hb].T.astype(BF16NP)
    d2 = np.zeros((128, 128), BF16NP)
    for q in range(4):
        d2[32 * q + 0, 0:64] = 1.0
        d2[32 * q + 1, 64:128] = 1.0
    if xt_cache is not None and b in xt_cache:
        xt, xt8 = xt_cache[b]
    else:
        xtf = make_xt(x[b])  # [128, 4, ntok] bf16
        if FP8_QUARTER:
            xt = np.ascontiguousarray(xtf[:, 1:4])
            ch0 = xtf[:, 0]  # [128 c, ntok]
            xt8 = np.ascontiguousarray(
                ch0.reshape(2, 64, -1).transpose(1, 0, 2)
            ).astype(F8NP)
        else:
            xt, xt8 = xtf, None
        if xt_cache is not None:
            xt_cache[b] = (xt, xt8)
    out = {
        "xT": xt,
        "w2": w2[:, :, 1:4] if FP8_QUARTER else w2,
        "c1": c1,
        "p2": p2,
        "d2": d2,
    }
    if FP8_QUARTER:
        # w28[ki, hp, t, j2] = w2[64t+ki, hp, 0, j2]
        out["xT8"] = xt8
        out["w28"] = np.ascontiguousarray(
            w2[:, :, 0].reshape(2, 64, n_hp, 128).transpose(1, 2, 0, 3)
        ).astype(F8NP)
    return out


_NC_CACHE = None
LAST_RESULTS = None


def kernel(**inputs):
    global _NC_CACHE, LAST_RESULTS
    from concourse.bass_utils import run_bass_kernel_spmd

    x = np.asarray(inputs["x"], np.float32)
    fw, fb, g, cb = fuse_weights(inputs)

    if _NC_CACHE is None:
        _NC_CACHE = build_bass()
    nc = _NC_CACHE

    xt_cache = {}
    in_maps = []
    for c in range(NCORES):
        b, gg = c // 2, c % 2
        in_maps.append(make_core_inputs(x, fw, fb, g, b, gg, xt_cache=xt_cache))

    res = run_bass_kernel_spmd(nc, in_maps, list(range(NCORES)))
    LAST_RESULTS = res

    out = np.empty((B, NTOK, DIM), np.float32)
    cbf = cb.astype(np.float32)
    for b in range(B):
        out[b] = res.results[2 * b]["out_p"].astype(np.float32)
        out[b] += res.results[2 * b + 1]["out_p"].astype(np.float32)
        out[b] += cbf[None, :]
    return out



# revision 80
# speedup vs baseline: 1.0271x; 1.0271x over previous
"""MultiHeadExternalAttention Trainium2 kernel (v3).

Math (reference):
  h = x @ trans_w.T + trans_b            [B,N,4096] -> heads [B,64,N,64]
  a = h @ lin0_w.T + lin0_b              per-head [B,64,N,64]
  a = softmax(a, axis=N)
  a = a / (1e-10 + a.sum(-1, keepdims))  double norm over j
  o = a @ lin1_w.T + lin1_b
  out = o (merged heads) @ proj_w.T + proj_b

Both tiny linears fold into the big matmuls on the host:
  logits[b,h,n,j] = x[b,n,:] @ fw[h,j,:] + fb[h,j]     fw = lin0_w @ trans_w_h
  out[b,n,c]     += attn[b,h,n,k] * g[h,c,k]           g  = proj_w_h @ lin1_w
  out[b,n,c]     += cb[c]                               (host)

Sharding: 8 cores = 4 batches x 2 head-halves (32 heads = 16 head-pairs per
core). v2: host-pretransposed bf16 xT; column-tiled jsum matmuls; DMA-based
r-broadcast; bf16 output partials.

v3 changes (TimelineSim 543 -> 517 us; each validated against the
device-occupancy simulator):
  - FOUR tail head-pairs stay SBUF-resident (ns = n_hp-4): the ep pool is a
    4-buffer ring and nothing re-allocates after e(12)..e(15), so those
    attn slabs survive into phase 2 for free. Spilling hp12/13 and
    gathering them back through the transition's saturated DMA window
    (sim showed ~25 us of queued transfers on the exclusive DMA engines)
    was pure waste: -8 MB DMA traffic, transition PE idle 12 -> 4.5 us.
  - a2 gathers split (hp 0..10 / hp 11) and prefetched two epochs before
    the transition, off the DMA crunch.
  - startup: hp0+hp1 logits emission token-interleaved so the PE stream
    matches x-chunk arrival order (hp-serial emission head-of-line blocked
    the PE on hp0's late windows: ~17 us idle).
  - softmax column scale rs folds into the phase-2 weights on device
    (p2' = rs*p2 per hp, 16 cheap DVE ops), so the attn scaling pass is a
    plain tensor_tensor (DVE 2x_1p mode, 297 ns/window) instead of
    scalar_tensor_tensor (no 2x support, 594 ns/window): DVE busy 197 ->
    132 us in sim, and half the STT time on HW.
  - attn spills to DRAM scratch go out piecewise (per 4 STT windows) on the
    sync queue: the a2 gathers depend only on the piece covering their
    tokens, and the Q7 descriptor-gen path is avoided.
  - the a2 gather is split (hp 0..12 / hp 13) so early-chunk matmuls don't
    wait on the last head-pair's spill.
  - transition: the first 2 phase-2 chunks emit their hp0..12 matmuls ahead
    of jsum(15) (whose Tmh dependency would head-of-line block the 4-deep
    PE wait queue), with the hp13/14/15 finishers deferred; the tail jsums
    run q-major ("pipelined") with per-q psum tiles from the idle p1p ring,
    per-q reciprocals, rdram stores and RB broadcast pieces, and the tail
    STT starts right behind them.
  - Tmh is allocated+cleared at slab start and jsum emission is deferred
    two s1 yields so its ldweights never clogs the PE wait queue.
  - p2 loads: slices 0:2 mid-x-stream, rest at epoch 2 (a bulk load would
    starve the startup x stream; per-hp slices later would pin DVE
    wait-queue slots on the in-flight rs folds).

Numerics note: RECIPROCAL_APPROX_FAST with a bf16 output produces NaNs on
HW (the f32-out assert in bass.py is load-bearing); the exact DVE
reciprocal stays.
"""

import sys

if "/opt/trn_rl_repo" not in sys.path:
    sys.path.insert(0, "/opt/trn_rl_repo")

import numpy as np
import ml_dtypes

import concourse.bass as bass
import concourse.bacc as bacc
import concourse.mybir as mybir
import concourse.tile as tile

BF16NP = ml_dtypes.bfloat16
F32 = mybir.dt.float32
BF = mybir.dt.bfloat16
AF = mybir.ActivationFunctionType

DIM = 512
HEADS = 64
K = 64
B = 4
NTOK = 8192
NCORES = 8
HPC = 16  # head pairs per core

USE_DMA_BCAST = True
# The bf16-out RECIPROCAL_APPROX_FAST produces NaNs on HW (the f32-out
# assert in bass.py is load-bearing); keep the exact DVE reciprocal.
USE_APPROX_RECIP = False
PREF_N = 3
A2P_N = 3
JSP_N = 1
# Compute the first 128 of the 512-wide logits contraction as one fp8
# DoubleRow matmul (2x64 packed rows): 3 bf16 + 1 DR matmul per window
# instead of 4 bf16. Numerically fine (1.49e-2 vs the 2e-2 gate) but the
# DoubleRow weight loads defeat the LDW pipelining on HW: measured 589 us
# vs 471 us without. Keep off.
FP8_QUARTER = False
F8 = mybir.dt.float8e4
F8NP = ml_dtypes.float8_e4m3


def build_bass(ntok=NTOK, n_hp=HPC, reps=1):
    nc = bacc.Bacc()
    NCC = 3 if FP8_QUARTER else 4
    xT_d = nc.dram_tensor("xT", [128, NCC, ntok], BF, kind="ExternalInput")
    w2 = nc.dram_tensor("w2", [128, n_hp, NCC, 128], BF, kind="ExternalInput")
    if FP8_QUARTER:
        xT8_d = nc.dram_tensor("xT8", [64, 2, ntok], F8, kind="ExternalInput")
        w28 = nc.dram_tensor("w28", [64, n_hp, 2, 128], F8, kind="ExternalInput")
    else:
        xT8_d = w28 = None
    c1 = nc.dram_tensor("c1", [128, n_hp], F32, kind="ExternalInput")
    p2 = nc.dram_tensor("p2", [128, n_hp, DIM], BF, kind="ExternalInput")
    d2 = nc.dram_tensor("d2", [128, 128], BF, kind="ExternalInput")
    out_p = nc.dram_tensor("out_p", [ntok, DIM], BF, kind="ExternalOutput")

    with tile.TileContext(nc) as tc:
        with tc.tile_pool(name="const", bufs=1) as const, tc.tile_pool(
            name="dramp", bufs=1, space="DRAM"
        ) as dramp:
            scratch = dramp.tile([max(n_hp - 2, 1), 128, ntok], BF)

            w2_sb = const.tile([128, n_hp, NCC, 128], BF)
            c1_sb = const.tile([128, n_hp], F32)
            p2_sb = const.tile([128, n_hp, DIM], BF)
            xT = const.tile([128, NCC, ntok], BF)
            if FP8_QUARTER:
                w28_sb = const.tile([64, n_hp, 2, 128], F8)
                xT8 = const.tile([64, 2, ntok], F8)
            else:
                w28_sb = xT8 = None
            D2 = const.tile([128, 128], BF)
            if not USE_DMA_BCAST:
                nc.scalar.dma_start(out=D2, in_=d2[:])

            for _rep in range(reps):
                run_pipeline(
                    nc, tc, xT_d, out_p, scratch, w2_sb, c1_sb, p2_sb, xT, D2,
                    dramp, ntok, n_hp,
                    (w2, c1, p2, w28) if _rep == 0 else None,
                    xT8_d=xT8_d, w28_sb=w28_sb, xT8=xT8,
                )
    nc.finalize()
    return nc


def run_pipeline(
    nc, tc, xT_d, out_p, scratch, w2_sb, c1_sb, p2_sb, xT, D2, dramp, ntok,
    n_hp, dram_consts=None, xT8_d=None, w28_sb=None, xT8=None,
):
    NT = ntok // 128
    NW = ntok // 512
    Q = min(4, NW)
    NS = NW // Q

    if dram_consts is not None:
        w2d, c1d, p2d, w28d = dram_consts
        # hp0 weight slice + bias first so the first logits matmul waits
        # only on 128KB of weights + one x chunk.
        nc.scalar.dma_start(out=w2_sb[:, 0:1], in_=w2d[:, 0:1])
        nc.scalar.dma_start(out=w2_sb[:, 1:2], in_=w2d[:, 1:2])
        if FP8_QUARTER:
            nc.scalar.dma_start(out=w28_sb, in_=w28d[:])
        nc.scalar.dma_start(out=c1_sb, in_=c1d[:])
    # x load in chunks so phase 1 can start early (deps are AP-granular).
    # All x loads on the sync HWDGE queue. Measured dead ends: gpsimd
    # routing for rep>=2 (Q7 descriptor-gen ~9us per 512-descriptor strided
    # chunk: 471->608us), scalar routing (x queues behind the final out
    # stores, no overlap gained).
    xq = nc.sync
    NXCH = 16
    for i in range(NXCH):
        lo, hi = ntok * i // NXCH, ntok * (i + 1) // NXCH
        xq.dma_start(out=xT[:, :, lo:hi], in_=xT_d[:, :, lo:hi])
        if FP8_QUARTER:
            xq.dma_start(out=xT8[:, :, lo:hi], in_=xT8_d[:, :, lo:hi])
        if i == 0 and dram_consts is not None:
            nc.scalar.dma_start(out=w2_sb[:, 2:n_hp], in_=w2d[:, 2:n_hp])
        if i == 11 and dram_consts is not None:
            # p2 slices for hp0/1 must be resident before emit_jsum folds rs
            # into them at the end of the interleaved startup (~13us); the
            # rest loads after the x stream so it can't starve it
            nc.scalar.dma_start(out=p2_sb[:, 0:2], in_=p2d[:, 0:2])

    # 4 tail head-pairs stay SBUF-resident: the ep pool is a 4-buffer ring
    # and nothing re-allocates after e(n_hp-4)..e(n_hp-1), so those slabs
    # survive into phase 2 for free — spilling them (and gathering them
    # back through the transition's saturated DMA window) was pure waste.
    ns = max(n_hp - 4, 0)
    tail_attn = {}
    # PSUM budget is 8 banks: p1p + jsp + op + (scp only for the PE-bcast
    # fallback; with DMA bcast its banks go to deeper logits buffering).
    n_p1 = 4 if USE_DMA_BCAST else 3
    import contextlib
    scp_ctx = (
        contextlib.nullcontext(None)
        if USE_DMA_BCAST
        else tc.tile_pool(name="scp", bufs=2, space="PSUM")
    )
    with tc.tile_pool(
        name="p1p", bufs=n_p1, space="PSUM"
      ) as p1p, tc.tile_pool(
        name="jsp", bufs=JSP_N, space="PSUM"
      ) as jsp, scp_ctx as scp, tc.tile_pool(name="ep", bufs=4) as ep, tc.tile_pool(
        name="rp", bufs=2
      ) as rp, tc.tile_pool(name="tmp", bufs=2) as tmp, tc.tile_pool(
        name="small", bufs=4
      ) as small, tc.tile_pool(name="a2p", bufs=A2P_N) as a2p, tc.tile_pool(
        # op stays at 2 psum banks: 3 banks measured 582us vs 471us (the
        # documented psum-bank-cycling HAM oscillation trap)
        name="op", bufs=2, space="PSUM"
      ) as op, tc.tile_pool(name="osp", bufs=2) as osp, tc.tile_pool(
        name="rbp", bufs=2
      ) as rbp:
        state = {}

        def gen_step1(hp, ygran=4, w28_sb=w28_sb, xT8=xT8):
            e = ep.tile([128, ntok], BF, name="e")
            scol = small.tile([128, NW], F32, name="scol")
            state[hp] = [e, None, None]
            # allocate + clear Tmh up front: only the rs copies depend on
            # the finished slab, so the DVE memset is long done by the time
            # emit_jsum's ldweights needs the tile (PE wait-queue is only 4
            # deep — a not-ready lhsT clogs it for unrelated matmuls).
            Tmh = tmp.tile([128, 66], BF, name="Tmh")
            nc.vector.memset(Tmh, 0.0)
            NCC = 3 if FP8_QUARTER else 4
            for t8 in range(NW):
                p1 = p1p.tile([128, 512], F32, name="p1")
                base = 512 * t8
                if FP8_QUARTER:
                    nc.tensor.matmul(
                        p1,
                        lhsT=w28_sb[:, hp],
                        rhs=xT8[:, :, base : base + 512],
                        perf_mode=mybir.MatmulPerfMode.DoubleRow,
                        start=True,
                        stop=False,
                    )
                for cc in range(NCC):
                    nc.tensor.matmul(
                        p1,
                        lhsT=w2_sb[:, hp, cc, :],
                        rhs=xT[:, cc, base : base + 512],
                        start=(cc == 0 and not FP8_QUARTER),
                        stop=(cc == NCC - 1),
                    )
                nc.scalar.activation(
                    e[:, base : base + 512],
                    p1,
                    func=AF.Exp,
                    bias=c1_sb[:, hp : hp + 1],
                    scale=1.0,
                    accum_out=scol[:, t8 : t8 + 1],
                )
                if t8 % ygran == ygran - 1:
                    yield
            s1 = small.tile([128, 1], F32, name="s1")
            nc.vector.reduce_sum(s1, scol, axis=mybir.AxisListType.X)
            rs = small.tile([128, 1], F32, name="rs")
            nc.vector.reciprocal(rs, s1)
            nc.vector.tensor_copy(Tmh[0:64, 32:33], rs[0:64, :])
            nc.vector.tensor_copy(Tmh[64:128, 33:34], rs[64:128, :])
            state[hp][1] = rs
            state[hp][2] = Tmh

        def emit_jsum(hp, pipelined=False):
            e, rs, Tmh = state[hp]
            assert Tmh is not None
            # 4-way column-tiled partition sums: window w = NS*q + s runs in
            # col-group q, accumulating at psum rows 32q + 2s + g. Body hps
            # emit s-major (consecutive matmuls in distinct col groups run
            # concurrently, recips/stores batched after); the tail hps emit
            # q-major (pipelined=True) so group q's accumulation closes
            # after its 4 matmuls, letting recip(q) -> rdram(q) -> RB
            # piece(q) -> STT windows 4q..4q+4 pipeline against group q+1's
            # matmuls — the whole r apparatus for the last hp sits on the
            # phase-1 -> phase-2 critical path.
            # pipelined mode allocates one PSUM tile per q-group from p1p
            # (idle by the transition): with a single shared js tile the
            # dependency tracking serializes group q+1's matmuls behind
            # recip(q)'s read (tile-granular WAR).
            js = None if pipelined else jsp.tile([128, 512], F32, name="js")
            jrb = rp.tile([128, 512], BF, name="jrb")
            # rdram row = 2w + g (window-ordered: rows 8q+2s+g, w = NS*q+s)
            rdram = dramp.tile([2 * NW, 512], BF, name="rdram", bufs=2)
            RB = rbp.tile([128, NW, 512], BF, name="RB") if USE_DMA_BCAST else None
            r2a = None if USE_DMA_BCAST else rp.tile([128, NS, 512], BF, name="r2a")
            rvb = rdram.rearrange("(w g) n -> g w n", w=NW, g=2)
            rva = rdram.rearrange("(k q g) n -> q g k n", k=NS, q=Q)
            from concourse.dve_ops import (
                RECIP_APPROX_FAST_CONSTS,
                RECIPROCAL_APPROX_FAST,
            )

            cns = RECIP_APPROX_FAST_CONSTS
            if not pipelined:
                for s in range(NS):
                    for q in range(Q):
                        w = NS * q + s
                        nc.tensor.matmul(
                            js[32 * q : 32 * q + 32, :],
                            lhsT=Tmh[:, 32 - 2 * s : 64 - 2 * s],
                            rhs=e[:, 512 * w : 512 * (w + 1)],
                            start=(s == 0),
                            stop=(s == NS - 1),
                            tile_position=(0, 32 * q),
                        )
            for q in range(Q):
                if pipelined:
                    js = p1p.tile([128, 512], F32, name="p1")
                    for s in range(NS):
                        w = NS * q + s
                        nc.tensor.matmul(
                            js[32 * q : 32 * q + 32, :],
                            lhsT=Tmh[:, 32 - 2 * s : 64 - 2 * s],
                            rhs=e[:, 512 * w : 512 * (w + 1)],
                            start=(s == 0),
                            stop=(s == NS - 1),
                            tile_position=(0, 32 * q),
                        )
                with nc.allow_low_precision(reason="r broadcast is bf16 either way"):
                    if USE_APPROX_RECIP:
                        # ~51-ULP approximate reciprocal (1 DVE op, ~1
                        # cyc/elem vs 6 for the exact iterative divide on
                        # HW). Emitted via _custom_dve to allow the bf16
                        # output: the fp32 requirement is about the
                        # BITWISE_NOT seed reading fp32 bits — the INPUT
                        # (psum f32) — while the write port downcasts like
                        # any DVE op. Denominators are sums of positives, far
                        # from the undefined 0/denorm/inf edges.
                        nc.vector._custom_dve(
                            RECIPROCAL_APPROX_FAST,
                            out=jrb[32 * q : 32 * q + 2 * NS, :],
                            in0=js[32 * q : 32 * q + 2 * NS, :],
                            s0=cns["s0"],
                            s1=cns["s1"],
                            imm2=cns["imm2"],
                        )
                    else:
                        nc.vector.reciprocal(
                            jrb[32 * q : 32 * q + 2 * NS, :],
                            js[32 * q : 32 * q + 2 * NS, :],
                        )
                nc.scalar.dma_start(
                    out=rdram[2 * NS * q : 2 * NS * (q + 1), :],
                    in_=jrb[32 * q : 32 * q + 2 * NS, :],
                )
                if USE_DMA_BCAST:
                    # replicating read: RB[64g+k, w, nn] = rdram[2w+g, nn],
                    # piece q covers windows 4q..4q+4 (rdram rows 8q..8q+8,
                    # exactly the q-store above). sync-queue HWDGE: the
                    # descriptor expansion would swamp the gpsimd Q7
                    # generator, and the scalar queue head-of-line blocks the
                    # exp activations behind the rdram wait.
                    for g in range(2):
                        nc.sync.dma_start(
                            out=RB[64 * g : 64 * (g + 1), NS * q : NS * (q + 1), :],
                            in_=rvb[g][NS * q : NS * (q + 1)].partition_broadcast(64),
                        )
                else:
                    # pack window w at partitions 32*(w%Q)+{0,1}, free slot
                    # w//Q
                    nc.sync.dma_start(
                        out=r2a[32 * q : 32 * q + 2, :, :], in_=rva[q]
                    )
            # fold the softmax column scale rs into this hp's phase-2
            # weights: attn is then stored as e*rjs (a plain tensor_tensor,
            # which the DVE runs in 2x mode — scalar_tensor_tensor cannot)
            # and out = sum_j (e*rjs)[j,n] * (rs_j*p2[j,c]) is unchanged.
            nc.vector.tensor_scalar(
                out=p2_sb[:, hp, :],
                in0=p2_sb[:, hp, :],
                scalar1=rs,
                scalar2=None,
                op0=mybir.AluOpType.mult,
            )
            state[hp] = [e, rs, RB if USE_DMA_BCAST else r2a]

        def gen_stt(hp):
            e, rs, rmat = state.pop(hp)
            attn = e  # in-place: attn overwrites the e slab window by window
            if hp >= ns:
                tail_attn[hp] = attn
            for w in range(NW):
                if USE_DMA_BCAST:
                    in1 = rmat[:, w, :]
                else:
                    q = w % Q
                    sc = scp.tile([128, 512], F32, name="sc")
                    nc.tensor.matmul(
                        sc,
                        lhsT=D2[32 * q : 32 * q + 2, :],
                        rhs=rmat[32 * q : 32 * q + 2, w // Q, :],
                        start=True,
                        stop=True,
                        tile_position=(32 * q, 0),
                    )
                    in1 = sc
                nc.vector.tensor_tensor(
                    out=attn[:, 512 * w : 512 * (w + 1)],
                    in0=e[:, 512 * w : 512 * (w + 1)],
                    in1=in1,
                    op=mybir.AluOpType.mult,
                )
                if w % 4 == 3:
                    # piecewise spill on the sync HWDGE queue: each a2 gather
                    # for token chunk i then depends only on the piece
                    # covering its tokens, not the whole-slab spill — keeps
                    # the phase-1 -> phase-2 transition off the Q7
                    # descriptor-gen path and off the whole-slab dependency.
                    if hp < ns:
                        lo, hi = 512 * (w - 3), 512 * (w + 1)
                        nc.sync.dma_start(
                            out=scratch[hp][:, lo:hi], in_=attn[:, lo:hi]
                        )
                    yield

        # phase 2: out[n,c] = sum_hp attn_hp[:, chunk].T @ p2_hp
        PREF = PREF_N

        def issue_a2_a(i):
            # part A: hp 0..ns-2, spilled early — issued one epoch before
            # the transition so the big gathers (~2.4us each on the
            # exclusive DMA engines) are out of the transition's DMA crunch
            a2 = a2p.tile([128, ns, 128], BF, name="a2")
            if ns > 1:
                nc.sync.dma_start(
                    out=a2[:, 0 : ns - 1],
                    in_=scratch[
                        0 : ns - 1, :, 128 * i : 128 * (i + 1)
                    ].rearrange("h p n -> p h n"),
                )
            return a2

        def issue_a2_b(i, a2):
            # part B: hp ns-1, whose spill lands last. MUST be emitted after
            # that spill's first piece (same sync queue: a not-ready DMA at
            # the queue head would deadlock against the piece queued behind
            # it).
            nc.sync.dma_start(
                out=a2[:, ns - 1 : ns],
                in_=scratch[
                    ns - 1 : ns, :, 128 * i : 128 * (i + 1)
                ].rearrange("h p n -> p h n"),
            )

        def issue_a2(i):
            a2 = issue_a2_a(i)
            issue_a2_b(i, a2)
            return a2

        def gen_phase2_prefetch_a(a2s):
            if ns > 1:
                for i in range(min(PREF, NT)):
                    a2s[i] = issue_a2_a(i)
                    yield

        def gen_phase2_prefetch(a2s):
            if ns > 0:
                for i in range(min(PREF, NT)):
                    if i not in a2s:
                        a2s[i] = issue_a2_a(i)
                    issue_a2_b(i, a2s[i])
                    yield

        # first S2 chunks are emitted in two parts: the hp0..n_hp-2 matmuls
        # go into the PE stream BEFORE the last hp's jsum (they only need a2
        # + the first STT window of tail hp n_hp-2, all ready while the last
        # hp's r apparatus resolves), and the hp n_hp-1 finisher is emitted
        # with the tail STT. This keeps the PE fed across the phase-1 ->
        # phase-2 transition instead of head-of-line blocking on jsum(last).
        S2 = 2 if (ns >= 2 and n_hp >= 4) else 0
        p2heads = {}

        def gen_phase2_head(a2s):
            for i in range(S2):
                a2 = a2s.pop(i)
                po = op.tile([128, 512], F32, name="po")
                # hp 0..ns-2 only: everything already spilled by the end of
                # epoch n_hp-1 (the late hp ns-1 spill and the tail STTs are
                # covered by the finisher)
                for hpi in range(ns - 1):
                    nc.tensor.matmul(
                        po,
                        lhsT=a2[:, hpi, :],
                        rhs=p2_sb[:, hpi, :],
                        start=(hpi == 0),
                        stop=False,
                    )
                p2heads[i] = (po, a2)
                yield

        def gen_phase2(a2s):
            for i in range(NT):
                if i in p2heads:
                    po, a2 = p2heads.pop(i)
                    for hpi in range(ns - 1, n_hp):
                        if hpi < ns:
                            lhsT = a2[:, hpi, :]
                        else:
                            lhsT = tail_attn[hpi][:, 128 * i : 128 * (i + 1)]
                        nc.tensor.matmul(
                            po,
                            lhsT=lhsT,
                            rhs=p2_sb[:, hpi, :],
                            start=False,
                            stop=(hpi == n_hp - 1),
                        )
                else:
                    if ns > 0:
                        a2 = a2s.pop(i)
                    po = op.tile([128, 512], F32, name="po")
                    for hpi in range(n_hp):
                        if hpi < ns:
                            lhsT = a2[:, hpi, :]
                        else:
                            lhsT = tail_attn[hpi][:, 128 * i : 128 * (i + 1)]
                        nc.tensor.matmul(
                            po,
                            lhsT=lhsT,
                            rhs=p2_sb[:, hpi, :],
                            start=(hpi == 0),
                            stop=(hpi == n_hp - 1),
                        )
                if ns > 0 and i + PREF < NT:
                    # emitted after this chunk's matmuls: the new a2 reuses
                    # the buffer those matmuls are still reading
                    a2s[i + PREF] = issue_a2(i + PREF)
                osb = osp.tile([128, DIM], BF, name="osb")
                nc.scalar.activation(osb, po, func=AF.Copy)
                nc.scalar.dma_start(
                    out=out_p[128 * i : 128 * (i + 1), :], in_=osb
                )
                yield

        a2s = {}
        stt_started = {}
        p2preA = gen_phase2_prefetch_a(a2s)
        p2pre = gen_phase2_prefetch(a2s)
        p2head = gen_phase2_head(a2s)
        p2g = gen_phase2(a2s)
        # Startup: token-interleaved emission of hp0+hp1 logits so the PE
        # instruction stream consumes windows in x-chunk arrival order
        # (hp-serial emission head-of-line-blocks the PE on hp0's late
        # windows while hp1's early windows already have data).
        if n_hp >= 2:
            g0, g1 = gen_step1(0, ygran=1), gen_step1(1, ygran=1)
            d0 = d1 = False
            while not (d0 and d1):
                if not d0 and next(g0, "END") == "END":
                    d0 = True
                    emit_jsum(0)
                if not d1 and next(g1, "END") == "END":
                    d1 = True
            hp_start = 2
        else:
            hp_start = 0
        for hp in range(hp_start, n_hp + 2):
            if hp == hp_start and dram_consts is not None:
                # the rest of p2 loads once the x stream is done competing
                # for the DMA engines (needed from jsum(2) ~epoch 3 on)
                nc.scalar.dma_start(out=p2_sb[:, 2:n_hp], in_=dram_consts[2][:, 2:n_hp])
            s1g = gen_step1(hp) if hp < n_hp else None
            # countdown: emit jsum(hp-1) only after two s1 yields (~8 logits
            # windows), by which point the DVE has drained the Tmh rs-copies
            # — otherwise the jsum matmuls sit not-ready in the 4-deep PE
            # wait queue and block the logits stream behind them.
            jsum_count = 2 if max(hp_start, 1) <= hp <= n_hp else -1
            if hp == n_hp and S2 > 0:
                # transition epoch: the S2 chunk heads were already emitted
                # at the end of the previous epoch; start the tail STT
                # (registers its slab + emits window 0) right behind
                # jsum(last).
                bcg = gen_stt(hp - 2)
                next(bcg, None)
                emit_jsum(hp - 1, pipelined=True)
                jsum_count = -1
                # start the tail STT immediately so its first windows land in
                # the DVE queue ahead of STT(hp-2)'s remaining windows (the
                # chunk finishers and early full chunks only need tail
                # windows 0..3)
                g_tail = gen_stt(hp - 1)
                next(g_tail, None)
                stt_started[hp - 1] = g_tail
            else:
                if jsum_count >= 0 and s1g is None:
                    emit_jsum(hp - 1, pipelined=(hp - 1 >= n_hp - 2))
                    jsum_count = -1
                if hp - 2 in stt_started:
                    bcg = stt_started.pop(hp - 2)
                else:
                    bcg = gen_stt(hp - 2) if hp >= 2 else None
            # prefetch epoch starts one earlier than phase 2: a2 gathers
            # interleave with the STT spill pieces they depend on; last
            # epoch: full phase-2 chunks.
            preA = hp >= n_hp - 2
            # all spills (hp < ns = n_hp-4) complete two epochs before the
            # transition, so the full prefetch can run at n_hp-2 as well
            pre, tail = hp >= n_hp - 2, hp == n_hp + 1
            while s1g is not None or bcg is not None:
                if s1g is not None and next(s1g, "END") == "END":
                    s1g = None
                if jsum_count >= 0:
                    jsum_count -= 1
                    if jsum_count <= 0 or s1g is None:
                        emit_jsum(hp - 1, pipelined=(hp - 1 >= n_hp - 2))
                        jsum_count = -1
                if bcg is not None and next(bcg, "END") == "END":
                    bcg = None
                if preA:
                    next(p2preA, None)
                if pre:
                    next(p2pre, None)
                if tail:
                    next(p2g, None)
            if hp == n_hp - 1 and S2 > 0:
                # emit the S2 chunk heads (hp 0..ns-2 matmuls) at the end of
                # this epoch: their a2 gathers and spills are complete, so
                # they give the PE covering work while the last hp's jsum/r
                # apparatus resolves next epoch
                for _ in range(S2):
                    next(p2pre, None)  # ensure a2[0..S2-1] issued
                for _ in range(S2):
                    next(p2head, None)
        for _ in p2preA:
            pass
        for _ in p2pre:
            pass
        for _ in p2head:
            pass
        for _ in p2g:
            pass
    tail_attn.clear()


def fuse_weights(inputs):
    tw = np.asarray(inputs["trans_w"], np.float64)  # [4096, 512]
    tb = np.asarray(inputs["trans_b"], np.float64)  # [4096]
    l0w = np.asarray(inputs["lin0_w"], np.float64)  # [64, 64]
    l0b = np.asarray(inputs["lin0_b"], np.float64)
    l1w = np.asarray(inputs["lin1_w"], np.float64)
    l1b = np.asarray(inputs["lin1_b"], np.float64)
    pw = np.asarray(inputs["proj_w"], np.float64)  # [512, 4096]
    pb = np.asarray(inputs["proj_b"], np.float64)

    tw3 = tw.reshape(HEADS, K, DIM)
    tb2 = tb.reshape(HEADS, K)
    fw = np.einsum("jk,hkc->hjc", l0w, tw3)  # [64, 64, 512]
    fb = l0b[None, :] + np.einsum("jk,hk->hj", l0w, tb2)  # [64, 64]
    pw3 = pw.reshape(DIM, HEADS, K).transpose(1, 0, 2)  # [h, c, j]
    g = np.einsum("hcj,jk->hck", pw3, l1w)  # [64, 512, 64]
    cb = pb + np.einsum("hcj,j->c", pw3, l1b)  # [512]
    return fw, fb, g, cb


def make_xt(xb):
    """xT[128, 4, ntok] bf16 from x[b] [ntok, DIM] f32."""
    ntok = xb.shape[0]
    xt = np.asarray(xb, np.float32).T.astype(BF16NP)  # [512, ntok]
    return np.ascontiguousarray(
        xt.reshape(4, 128, ntok).transpose(1, 0, 2)
    )


def make_core_inputs(x, fw, fb, g, b, gg, n_hp=HPC, xt_cache=None):
    """Inputs for the core handling batch b, head half gg (heads 32*gg..+32)."""
    h0 = (HEADS // 2) * gg
    w2 = np.empty((128, n_hp, 4, 128), BF16NP)
    c1 = np.empty((128, n_hp), np.float32)
    p2 = np.empty((128, n_hp, DIM), BF16NP)
    for hp in range(n_hp):
        ha, hb = h0 + 2 * hp, h0 + 2 * hp + 1
        blk = np.concatenate([fw[ha], fw[hb]], axis=0)  # [128 j2, 512 c]
        # w2[ci, hp, cc, j2] = blk[j2, cc*128+ci]
        w2[:, hp, :, :] = blk.reshape(128, 4, 128).transpose(2, 1, 0).astype(BF16NP)
        c1[:, hp] = np.concatenate([fb[ha], fb[hb]]).astype(np.float32)
        # p2[g2*64+k, hp, c] = g[head, c, k]
        p2[0:64, hp, :] = g[ha].T.astype(BF16NP)
        p2[64:128, hp, :] = g[hb].T.astype(BF16NP)
    d2 = np.zeros((128, 128), BF16NP)
    for q in range(4):
        d2[32 * q + 0, 0:64] = 1.0
        d2[32 * q + 1, 64:128] = 1.0
    if xt_cache is not None and b in xt_cache:
        xt, xt8 = xt_cache[b]
    else:
        xtf = make_xt(x[b])  # [128, 4, ntok] bf16
        if FP8_QUARTER:
            xt = np.ascontiguousarray(xtf[:, 1:4])
            ch0 = xtf[:, 0]  # [128 c, ntok]
            xt8 = np.ascontiguousarray(
                ch0.reshape(2, 64, -1).transpose(1, 0, 2)
            ).astype(F8NP)
        else:
            xt, xt8 = xtf, None
        if xt_cache is not None:
            xt_cache[b] = (xt, xt8)
    out = {
        "xT": xt,
        "w2": w2[:, :, 1:4] if FP8_QUARTER else w2,
        "c1": c1,
        "p2": p2,
        "d2": d2,
    }
    if FP8_QUARTER:
        # w28[ki, hp, t, j2] = w2[64t+ki, hp, 0, j2]
        out["xT8"] = xt8
        out["w28"] = np.ascontiguousarray(
            w2[:, :, 0].reshape(2, 64, n_hp, 128).transpose(1, 2, 0, 3)
        ).astype(F8NP)
    return out


_NC_CACHE = None
LAST_RESULTS = None


def kernel(**inputs):
    global _NC_CACHE, LAST_RESULTS
    from concourse.bass_utils import run_bass_kernel_spmd

    x = np.asarray(inputs["x"], np.float32)
    fw, fb, g, cb = fuse_weights(inputs)

    if _NC_CACHE is None:
        _NC_CACHE = build_bass()
    nc = _NC_CACHE

    xt_cache = {}
    in_maps = []
    for c in range(NCORES):
        b, gg = c // 2, c % 2
        in_maps.append(make_core_inputs(x, fw, fb, g, b, gg, xt_cache=xt_cache))

    res = run_bass_kernel_spmd(nc, in_maps, list(range(NCORES)))
    LAST_RESULTS = res

    out = np.empty((B, NTOK, DIM), np.float32)
    cbf = cb.astype(np.float32)
    for b in range(B):
        out[b] = res.results[2 * b]["out_p"].astype(np.float32)
        out[b] += res.results[2 * b + 1]["out_p"].astype(np.float32)
        out[b] += cbf[None, :]
    return out

